# revision 1
# baseline (speedup 1.0000x reference)
"""Causal multi-head self-attention with RoPE on 8 NeuronCores.

Sharding: batch (4) x head-halves (2) -> 8 cores. Core c handles batch c//2,
heads [8*(c%2), 8*(c%2)+8). QKV/attention are computed per-core in transposed
layout (dims on partitions, sequence on free dim); the output projection is
row-sharded over Wo with a pair AllReduce producing the full output on the
even core of each pair.
"""

import numpy as np
import ml_dtypes

import concourse.bacc as bacc
import concourse.bass as bass
import concourse.mybir as mybir
from concourse.tile import TileContext
from concourse.bass_utils import run_bass_kernel_spmd

B, S, D, H = 4, 2048, 1024, 16
HL = 8          # heads per core
DK = 64         # head dim
NCORES = 8
DT = D // 128   # 8 d-tiles (contraction tiles)
OT = HL * DK // 128   # 4 o-tiles for Q^T/K^T ([128, S] each, 2 heads per tile)
ST = S // 128   # 16 s-tiles
NCH = S // 512  # 4 sequence chunks of 512
VW = DK + 1     # V columns per head incl. ones column

BF16 = mybir.dt.bfloat16
F32 = mybir.dt.float32
NEG = -1.0e9

_compiled = {}


def _build_nc():
    nc = bacc.Bacc("TRN2", target_bir_lowering=False, debug=False,
                   num_devices=NCORES)

    xT = nc.dram_tensor("xT", [D, S], BF16, kind="ExternalInput")
    wqT = nc.dram_tensor("wqT", [D, HL * DK], BF16, kind="ExternalInput")
    wkT = nc.dram_tensor("wkT", [D, HL * DK], BF16, kind="ExternalInput")
    wvT = nc.dram_tensor("wvT", [D, HL * DK], BF16, kind="ExternalInput")
    woT = nc.dram_tensor("woT", [HL * DK, D], BF16, kind="ExternalInput")
    cosT = nc.dram_tensor("cosT", [128, S], BF16, kind="ExternalInput")
    sinT = nc.dram_tensor("sinT", [128, S], BF16, kind="ExternalInput")
    swapT = nc.dram_tensor("swapT", [128, 128], BF16, kind="ExternalInput")
    maskT = nc.dram_tensor("maskT", [128, 128], F32, kind="ExternalInput")
    y = nc.dram_tensor("y", [S, D], BF16, kind="ExternalOutput")

    groups = [[0, 1], [2, 3], [4, 5], [6, 7]]

    with TileContext(nc) as tc:
        with (
            tc.tile_pool(name="big", bufs=1) as big,
            tc.tile_pool(name="work", bufs=2) as work,
            tc.tile_pool(name="ptile", bufs=10) as ptile,
            tc.tile_pool(name="norm", bufs=2) as normp,
            tc.tile_pool(name="ps_b", bufs=3, space="PSUM") as ps_b,
            tc.tile_pool(name="ps_o", bufs=2, space="PSUM") as ps_o,
            tc.tile_pool(name="dram", bufs=1, space="DRAM") as dram,
        ):
            # ---- load constant-ish inputs ----
            w_sb = {"q": [], "k": [], "v": []}
            for name, drt in (("q", wqT), ("k", wkT), ("v", wvT)):
                for k in range(DT):
                    t = big.tile([128, HL * DK], BF16, tag=f"w{name}{k}", name=f"w{name}{k}")
                    nc.scalar.dma_start(out=t[:], in_=drt[128 * k:128 * (k + 1), :])
                    w_sb[name].append(t)
            xT_sb = []
            for k in range(DT):
                t = big.tile([128, S], BF16, tag=f"xT{k}", name=f"xT{k}")
                eng = nc.sync if k % 2 == 0 else nc.scalar
                eng.dma_start(out=t[:], in_=xT[128 * k:128 * (k + 1), :])
                xT_sb.append(t)
            woT_sb = []
            for k in range(OT):
                t = big.tile([128, D], BF16, tag=f"wo{k}", name=f"wo{k}")
                nc.sync.dma_start(out=t[:], in_=woT[128 * k:128 * (k + 1), :])
                woT_sb.append(t)
            cos_sb = big.tile([128, S], BF16, tag="cos")
            nc.scalar.dma_start(out=cos_sb[:], in_=cosT[:])
            sin_sb = big.tile([128, S], BF16, tag="sin")
            nc.scalar.dma_start(out=sin_sb[:], in_=sinT[:])
            swap_sb = big.tile([128, 128], BF16, tag="swap")
            nc.scalar.dma_start(out=swap_sb[:], in_=swapT[:])
            mask_sb = big.tile([128, 128], F32, tag="mask")
            nc.scalar.dma_start(out=mask_sb[:], in_=maskT[:])

            # ---- phase A: QKV projections + RoPE ----
            qrot_sb = [big.tile([128, S], BF16, tag=f"qr{t}", name=f"qr{t}") for t in range(OT)]
            krot_sb = [big.tile([128, S], BF16, tag=f"kr{t}", name=f"kr{t}") for t in range(OT)]
            vaug_sb = [big.tile([128, HL * VW], BF16, tag=f"va{st}", name=f"va{st}")
                       for st in range(ST)]

            for wname, rot_tiles in (("q", qrot_sb), ("k", krot_sb)):
                for t in range(OT):
                    raw = work.tile([128, S], BF16, tag="raw")
                    for n in range(NCH):
                        ps = ps_b.tile([128, 1024], F32, tag="psb", name="ps")[:, 0:512]
                        for k in range(DT):
                            nc.tensor.matmul(
                                ps[:],
                                lhsT=w_sb[wname][k][:, 128 * t:128 * (t + 1)],
                                rhs=xT_sb[k][:, 512 * n:512 * (n + 1)],
                                start=(k == 0), stop=(k == DT - 1),
                            )
                        nc.scalar.copy(out=raw[:, 512 * n:512 * (n + 1)], in_=ps[:])
                    t1 = work.tile([128, S], BF16, tag="t1", bufs=1)
                    nc.vector.tensor_mul(t1[:], raw[:], cos_sb[:])
                    t2 = work.tile([128, S], BF16, tag="t2", bufs=1)
                    for n in range(NCH):
                        sl = slice(512 * n, 512 * (n + 1))
                        ps2 = ps_b.tile([128, 1024], F32, tag="psb", name="ps2")[:, 0:512]
                        nc.tensor.matmul(ps2[:], lhsT=swap_sb[:], rhs=raw[:, sl],
                                         start=True, stop=True)
                        nc.vector.tensor_mul(t2[:, sl], ps2[:], sin_sb[:, sl])
                    nc.vector.tensor_add(rot_tiles[t][:], t1[:], t2[:])

            for st in range(ST):
                ps = ps_b.tile([128, 1024], F32, tag="psb", name="ps")[:, 0:512]
                for k in range(DT):
                    nc.tensor.matmul(
                        ps[:],
                        lhsT=xT_sb[k][:, 128 * st:128 * (st + 1)],
                        rhs=w_sb["v"][k][:],
                        start=(k == 0), stop=(k == DT - 1),
                    )
                # strided copy: head h cols [64h:64h+64] -> vaug cols [65h:65h+64]
                dst = vaug_sb[st][:].rearrange("p (h d) -> p h d", d=VW)[:, :, 0:DK]
                src = ps[:].rearrange("p (h d) -> p h d", d=DK)
                nc.vector.tensor_copy(dst, src)
                ones = vaug_sb[st][:].rearrange("p (h d) -> p h d", d=VW)[:, :, DK:VW]
                nc.gpsimd.memset(ones, 1.0)

            # ---- phase B/C: attention + projection + allreduce, per i-chunk ----
            ypart = dram.tile([S, D], BF16)
            yred = dram.tile([S, D], BF16)
            oT_sb = [big.tile([128, S], BF16, tag=f"oT{t}", name=f"oT{t}") for t in range(OT)]

            def attn_chunk(m):
                i0 = 512 * m
                njb = 4 * m + 4
                sums8 = normp.tile([HL, 512], F32, tag="sums8", bufs=1,
                                   name="sums8")
                o_sb = [None] * HL
                for tp in range(OT):  # head pair = o-tile tp (heads 2tp, 2tp+1)
                    o_pse = ps_o.tile([VW, 512], F32, tag="pso")
                    o_pso = ps_o.tile([VW, 512], F32, tag="pso")
                    pTs = [None] * njb
                    fulls = [jb for jb in range(njb) if 128 * jb < i0]
                    diags = [jb for jb in range(njb) if 128 * jb >= i0]
                    order = []
                    for idx in range(max(len(fulls), len(diags))):
                        if idx < len(diags):
                            order.append(diags[idx])
                        if idx < len(fulls):
                            order.extend(fulls[idx::4][:1])
                    # fallback: ensure every jb present exactly once
                    seen = set()
                    order = [jb for jb in order + list(range(njb))
                             if not (jb in seen or seen.add(jb))]
                    for jb in order:
                        j0 = 128 * jb
                        dlt = max(0, j0 - i0)
                        s_ps = ps_b.tile([128, 1024], F32, tag="psb")
                        for half, po in ((0, 0), (1, DK)):
                            nc.tensor.matmul(
                                s_ps[:, 512 * half + dlt:512 * (half + 1)],
                                lhsT=krot_sb[tp][po:po + DK, j0:j0 + 128],
                                rhs=qrot_sb[tp][po:po + DK, i0 + dlt:i0 + 512],
                                start=True, stop=True,
                            )
                        if j0 >= i0:
                            s3 = s_ps[:].rearrange("p (b f) -> p b f", b=2)
                            nc.vector.tensor_add(
                                s3[:, :, dlt:dlt + 128],
                                s3[:, :, dlt:dlt + 128],
                                mask_sb[:].rearrange("p (b f) -> p b f", b=1)
                                .broadcast_to([128, 2, 128]))
                        pT = ptile.tile([128, 1024], BF16, tag="pT")
                        nc.scalar.activation(
                            pT[:].rearrange("p (b f) -> p b f", b=2)[:, :, dlt:512],
                            s_ps[:].rearrange("p (b f) -> p b f", b=2)[:, :, dlt:512],
                            mybir.ActivationFunctionType.Exp, scale=0.125)
                        pTs[jb] = pT
                    for jb in range(njb):
                        dlt = max(0, 128 * jb - i0)
                        for half, o_ps in ((0, o_pse), (1, o_pso)):
                            nc.tensor.matmul(
                                o_ps[:, dlt:512],
                                lhsT=vaug_sb[jb][:, VW * (2 * tp + half):
                                                 VW * (2 * tp + half + 1)],
                                rhs=pTs[jb][:, 512 * half + dlt:512 * (half + 1)],
                                start=(jb == 0), stop=(jb == njb - 1),
                            )
                    for half, o_ps in ((0, o_pse), (1, o_pso)):
                        h = 2 * tp + half
                        osb = normp.tile([VW, 512], F32, tag=f"osb{h}", bufs=1,
                                         name=f"osb{h}")
                        nc.scalar.copy(out=osb[:], in_=o_ps[:])
                        nc.sync.dma_start(out=sums8[h:h + 1, :],
                                          in_=osb[DK:VW, :])
                        o_sb[h] = osb
                rec8 = normp.tile([HL, 512], F32, tag="rec8", bufs=1,
                                  name="rec8")
                nc.vector.reciprocal(rec8[:], sums8[:])
                for h in range(HL):
                    tp2, po = h // 2, DK * (h % 2)
                    stage = normp.tile([1, 512], F32, tag="stage", bufs=2,
                                       name="stage")
                    nc.sync.dma_start(out=stage[:], in_=rec8[h:h + 1, :])
                    rep = normp.tile([DK, 512], F32, tag="rep", bufs=2,
                                     name="rep")
                    nc.gpsimd.partition_broadcast(rep[:], stage[:])
                    nc.vector.tensor_mul(
                        oT_sb[tp2][po:po + DK, i0:i0 + 512],
                        o_sb[h][0:DK, :], rep[:])

            def proj_chunk(m, ncc):
                # projection of chunk m's rows (partial over my 512 dims)
                i0 = 512 * m
                for r2 in range(4):
                    r0 = i0 + 128 * r2
                    ych = work.tile([128, D], BF16, tag="ych")
                    for nn in range(2):
                        yp = ps_b.tile([128, 1024], F32, tag="psb", name="yp")[:, 0:512]
                        for k in range(OT):
                            nc.tensor.matmul(
                                yp[:],
                                lhsT=oT_sb[k][:, r0:r0 + 128],
                                rhs=woT_sb[k][:, 512 * nn:512 * (nn + 1)],
                                start=(k == 0), stop=(k == OT - 1),
                            )
                        nc.scalar.copy(out=ych[:, 512 * nn:512 * (nn + 1)], in_=yp[:])
                    nc.sync.dma_start(out=ypart[r0:r0 + 128, :], in_=ych[:])
                rows = 512 // ncc
                for q in range(ncc):
                    q0 = i0 + rows * q
                    nc.gpsimd.collective_compute(
                        "AllReduce", mybir.AluOpType.add, replica_groups=groups,
                        ins=[ypart[q0:q0 + rows, :].opt()],
                        outs=[yred[q0:q0 + rows, :].opt()],
                    )
                    nc.sync.dma_start(out=y[q0:q0 + rows, :],
                                      in_=yred[q0:q0 + rows, :])

            order_m = [3, 2, 1, 0]
            for idx, m in enumerate(order_m):
                attn_chunk(m)
                if idx >= 1:
                    proj_chunk(order_m[idx - 1], 1)
            proj_chunk(order_m[-1], 2)

    nc.compile()
    return nc


def _prep_inputs(x, Wq, Wk, Wv, Wo, cos_emb, sin_emb, token_positions):
    bf = ml_dtypes.bfloat16
    cos_g = np.asarray(cos_emb)[np.asarray(token_positions)]  # [S, DK]
    sin_g = np.asarray(sin_emb)[np.asarray(token_positions)]
    # [128, S]: partition p -> head-dim p % 64
    cosT = np.ascontiguousarray(np.tile(cos_g.T, (2, 1))).astype(bf)
    sinT = np.ascontiguousarray(np.tile(sin_g.T, (2, 1))).astype(bf)
    # rotate-half-interleaved as a matmul: rh = SWAP @ q (per 128-dim tile)
    swap = np.zeros((128, 128), np.float32)
    for j in range(64):
        swap[2 * j, 2 * j + 1] = -1.0
        swap[2 * j + 1, 2 * j] = 1.0
    swapT = np.ascontiguousarray(swap.T).astype(bf)
    # causal mask for the diagonal 128x128 block in S^T=[j,i] layout
    jj = np.arange(128)[:, None]
    ii = np.arange(128)[None, :]
    maskT = np.where(ii >= jj, 0.0, NEG).astype(np.float32)

    in_maps = []
    for c in range(NCORES):
        b, hh = c // 2, c % 2
        cols = slice(512 * hh, 512 * (hh + 1))
        in_maps.append({
            "xT": np.ascontiguousarray(np.asarray(x)[b].T).astype(bf),
            "wqT": np.ascontiguousarray(np.asarray(Wq)[cols, :].T).astype(bf),
            "wkT": np.ascontiguousarray(np.asarray(Wk)[cols, :].T).astype(bf),
            "wvT": np.ascontiguousarray(np.asarray(Wv)[cols, :].T).astype(bf),
            "woT": np.ascontiguousarray(np.asarray(Wo)[:, cols].T).astype(bf),
            "cosT": cosT, "sinT": sinT, "swapT": swapT, "maskT": maskT,
        })
    return in_maps


def kernel(x, Wq, Wk, Wv, Wo, cos_emb, sin_emb, token_positions, **run_kwargs):
    if "nc" not in _compiled:
        _compiled["nc"] = _build_nc()
    nc = _compiled["nc"]
    in_maps = _prep_inputs(x, Wq, Wk, Wv, Wo, cos_emb, sin_emb, token_positions)
    res = run_bass_kernel_spmd(nc, in_maps, list(range(NCORES)), **run_kwargs)
    out = np.stack([res.results[2 * b]["y"] for b in range(B)]).astype(np.float32)
    if run_kwargs:
        kernel.last_result = res
    return out



# revision 4
# speedup vs baseline: 1.1817x; 1.1817x over previous
"""Causal multi-head self-attention with RoPE on 8 NeuronCores.

Sharding: batch (4) x head-halves (2) -> 8 cores. Core c handles batch c//2,
heads [8*(c%2), 8*(c%2)+8). Pipeline: for each 512-row sequence chunk n
(ascending), QKV+RoPE for chunk n, then attention rows n against all j<=n,
then an AllGather of the bf16 o^T chunk across the core pair, then the
output projection column-sharded over Wo (each core produces y[:, 512 cols]).
Host concatenates the two column halves. No AllReduce.
"""

import numpy as np
import ml_dtypes

import concourse.bacc as bacc
import concourse.bass as bass
import concourse.mybir as mybir
from concourse.tile import TileContext
from concourse.bass_utils import run_bass_kernel_spmd

B, S, D, H = 4, 2048, 1024, 16
HL = 8          # heads per core
DK = 64         # head dim
NCORES = 8
DT = D // 128   # 8 d-tiles (contraction tiles)
OT = HL * DK // 128   # 4 o-tiles for Q^T/K^T ([128, S] each, 2 heads per tile)
ST = S // 128   # 16 s-tiles
NCH = S // 512  # 4 sequence chunks of 512
VW = 66         # V columns per head: 64 data + ones + zero pad (alignment)

BF16 = mybir.dt.bfloat16
F32 = mybir.dt.float32
NEG = -1.0e9

_compiled = {}


def _build_nc():
    nc = bacc.Bacc("TRN2", target_bir_lowering=False, debug=False,
                   num_devices=NCORES)

    xT = nc.dram_tensor("xT", [D, S], BF16, kind="ExternalInput")
    wqT = nc.dram_tensor("wqT", [D, HL * DK], BF16, kind="ExternalInput")
    wkT = nc.dram_tensor("wkT", [D, HL * DK], BF16, kind="ExternalInput")
    wvT = nc.dram_tensor("wvT", [D, HL * DK], BF16, kind="ExternalInput")
    woT2 = nc.dram_tensor("woT2", [D, 512], BF16, kind="ExternalInput")
    cosT = nc.dram_tensor("cosT", [128, S], BF16, kind="ExternalInput")
    sinT = nc.dram_tensor("sinT", [128, S], BF16, kind="ExternalInput")
    swapT = nc.dram_tensor("swapT", [128, 128], BF16, kind="ExternalInput")
    maskT = nc.dram_tensor("maskT", [128, 128], F32, kind="ExternalInput")
    y = nc.dram_tensor("y", [S, 512], BF16, kind="ExternalOutput")

    groups = [[0, 1], [2, 3], [4, 5], [6, 7]]

    with TileContext(nc) as tc:
        with (
            tc.tile_pool(name="big", bufs=1) as big,
            tc.tile_pool(name="work", bufs=2) as work,
            tc.tile_pool(name="ptile", bufs=8) as ptile,
            tc.tile_pool(name="norm", bufs=2) as normp,
            tc.tile_pool(name="ps_b", bufs=2, space="PSUM") as ps_b,
            tc.tile_pool(name="ps_sw", bufs=2, space="PSUM") as ps_sw,
            tc.tile_pool(name="ps_o", bufs=1, space="PSUM") as ps_o,
            tc.tile_pool(name="dram", bufs=1, space="DRAM") as dram,
        ):
            # ---- persistent SBUF tiles ----
            w_sb = {"q": [], "k": [], "v": []}
            for name, drt in (("q", wqT), ("k", wkT), ("v", wvT)):
                for k in range(DT):
                    t = big.tile([128, HL * DK], BF16, tag=f"w{name}{k}",
                                 name=f"w{name}{k}")
                    w_sb[name].append(t)
            wo_sb = [big.tile([128, 512], BF16, tag=f"wo{k}", name=f"wo{k}")
                     for k in range(DT)]
            xT_sb = [big.tile([128, S], BF16, tag=f"xT{k}", name=f"xT{k}")
                     for k in range(DT)]
            cos_sb = big.tile([128, S], BF16, tag="cos")
            sin_sb = big.tile([128, S], BF16, tag="sin")
            swap_sb = big.tile([128, 128], BF16, tag="swap")
            mask_sb = big.tile([128, 128], F32, tag="mask")
            qrot_sb = [big.tile([128, S], BF16, tag=f"qr{t}", name=f"qr{t}")
                       for t in range(OT)]
            krot_sb = [big.tile([128, S], BF16, tag=f"kr{t}", name=f"kr{t}")
                       for t in range(OT)]
            vaug_sb = [big.tile([128, HL * VW], BF16, tag=f"va{st}",
                                name=f"va{st}") for st in range(ST)]

            # ---- upfront DMA issuance, spread across queues ----
            nc.scalar.dma_start(out=swap_sb[:], in_=swapT[:])
            nc.scalar.dma_start(out=cos_sb[:], in_=cosT[:])
            nc.scalar.dma_start(out=sin_sb[:], in_=sinT[:])
            nc.scalar.dma_start(out=mask_sb[:], in_=maskT[:])
            for k in range(4):
                nc.scalar.dma_start(out=w_sb["q"][k][:],
                                    in_=wqT[128 * k:128 * (k + 1), :])
            for k in range(4, DT):
                nc.sync.dma_start(out=w_sb["q"][k][:],
                                  in_=wqT[128 * k:128 * (k + 1), :])
            for k in range(DT):
                eng = nc.sync if k % 2 == 0 else nc.gpsimd
                eng.dma_start(out=xT_sb[k][:, 0:512], in_=xT[128 * k:128 * (k + 1), 0:512])
            for k in range(DT):
                nc.gpsimd.dma_start(out=w_sb["k"][k][:],
                                    in_=wkT[128 * k:128 * (k + 1), :])
            for k in range(DT):
                nc.sync.dma_start(out=w_sb["v"][k][:],
                                  in_=wvT[128 * k:128 * (k + 1), :])
            for k in range(DT):
                nc.gpsimd.dma_start(out=wo_sb[k][:],
                                    in_=woT2[128 * k:128 * (k + 1), :])
            # ones / zero columns of vaug (static)
            for st in range(ST):
                v3 = vaug_sb[st][:].rearrange("p (h d) -> p h d", d=VW)
                nc.gpsimd.memset(v3[:, :, DK:DK + 1], 1.0)
                nc.gpsimd.memset(v3[:, :, DK + 1:VW], 0.0)

            # per-(chunk, head-pair) DRAM buffers for the o^T exchange
            ocp = [[dram.tile([128, 512], BF16, tag=f"ocp{n}_{tp}",
                              name=f"ocp{n}_{tp}")
                    for tp in range(OT)] for n in range(NCH)]
            ocg = [[dram.tile([256, 512], BF16, tag=f"ocg{n}_{tp}",
                              name=f"ocg{n}_{tp}")
                    for tp in range(OT)] for n in range(NCH)]

            def qkv_chunk(n):
                sl = slice(512 * n, 512 * (n + 1))
                # prefetch next chunk's xT columns
                if n + 1 < NCH:
                    sl2 = slice(512 * (n + 1), 512 * (n + 2))
                    for k in range(DT):
                        eng = nc.sync if k % 2 == 0 else nc.gpsimd
                        eng.dma_start(out=xT_sb[k][:, sl2],
                                      in_=xT[128 * k:128 * (k + 1), sl2])
                for wname, rot in (("q", qrot_sb), ("k", krot_sb)):
                    pss = []
                    for tpair in ((0, 1), (2, 3)):
                        ps = ps_b.tile([128, 1024], F32, tag="psb", name="ps")
                        pss.append(ps)
                        for k in range(DT):
                            for i, t in enumerate(tpair):
                                nc.tensor.matmul(
                                    ps[:, 512 * i:512 * (i + 1)],
                                    lhsT=w_sb[wname][k][:, 128 * t:128 * (t + 1)],
                                    rhs=xT_sb[k][:, sl],
                                    start=(k == 0), stop=(k == DT - 1),
                                )
                    for pi, tpair in enumerate(((0, 1), (2, 3))):
                        ps = pss[pi]
                        for i, t in enumerate(tpair):
                            psl = ps[:, 512 * i:512 * (i + 1)]
                            tmp = work.tile([128, 512], BF16, tag="tmp", name="tmp")
                            nc.vector.tensor_mul(tmp[:], psl, sin_sb[:, sl])
                            t1 = work.tile([128, 512], BF16, tag="t1", name="t1")
                            nc.vector.tensor_mul(t1[:], psl, cos_sb[:, sl])
                            ps2 = ps_sw.tile([128, 512], F32, tag="ps2", name="ps2")
                            nc.tensor.matmul(ps2[:], lhsT=swap_sb[:],
                                             rhs=tmp[:], start=True, stop=True)
                            nc.vector.tensor_add(rot[t][:, sl], t1[:], ps2[:])
                # V for the 4 s-tiles of this chunk
                for stp in ((4 * n, 4 * n + 1), (4 * n + 2, 4 * n + 3)):
                    ps = ps_b.tile([128, 1024], F32, tag="psb", name="psv")
                    for k in range(DT):
                        for i, st in enumerate(stp):
                            nc.tensor.matmul(
                                ps[:, 512 * i:512 * (i + 1)],
                                lhsT=xT_sb[k][:, 128 * st:128 * (st + 1)],
                                rhs=w_sb["v"][k][:],
                                start=(k == 0), stop=(k == DT - 1),
                            )
                    for i, st in enumerate(stp):
                        dst = vaug_sb[st][:].rearrange(
                            "p (h d) -> p h d", d=VW)[:, :, 0:DK]
                        src = ps[:, 512 * i:512 * (i + 1)].rearrange(
                            "p (h d) -> p h d", d=DK)
                        nc.vector.tensor_copy(dst, src)

            def attn_chunk(m):
                i0 = 512 * m
                njb = 4 * m + 4
                for tp in range(OT):  # head pair (heads 2tp, 2tp+1)
                    o_pse = ps_o.tile([VW, 512], F32, tag="psoe", name="o_pse")
                    o_pso = ps_o.tile([VW, 512], F32, tag="psoo", name="o_pso")
                    pTs = [None] * njb
                    fulls = [jb for jb in range(njb) if 128 * jb < i0]
                    diags = [jb for jb in range(njb) if 128 * jb >= i0]
                    order = []
                    for idx in range(max(len(fulls), len(diags))):
                        if idx < len(diags):
                            order.append(diags[idx])
                        if idx < len(fulls):
                            order.extend(fulls[idx::4][:1])
                    seen = set()
                    order = [jb for jb in order + list(range(njb))
                             if not (jb in seen or seen.add(jb))]
                    for jb in order:
                        j0 = 128 * jb
                        dlt = max(0, j0 - i0)
                        s_ps = ps_b.tile([128, 1024], F32, tag="psb", name="s_ps")
                        for half, po in ((0, 0), (1, DK)):
                            nc.tensor.matmul(
                                s_ps[:, 512 * half + dlt:512 * (half + 1)],
                                lhsT=krot_sb[tp][po:po + DK, j0:j0 + 128],
                                rhs=qrot_sb[tp][po:po + DK, i0 + dlt:i0 + 512],
                                start=True, stop=True,
                            )
                        if j0 >= i0:
                            s3 = s_ps[:].rearrange("p (b f) -> p b f", b=2)
                            nc.vector.tensor_add(
                                s3[:, :, dlt:dlt + 128],
                                s3[:, :, dlt:dlt + 128],
                                mask_sb[:].rearrange("p (b f) -> p b f", b=1)
                                .broadcast_to([128, 2, 128]))
                        pT = ptile.tile([128, 1024], BF16, tag="pT", name="pT")
                        nc.scalar.activation(
                            pT[:].rearrange("p (b f) -> p b f", b=2)[:, :, dlt:512],
                            s_ps[:].rearrange("p (b f) -> p b f", b=2)[:, :, dlt:512],
                            mybir.ActivationFunctionType.Exp, scale=0.125)
                        pTs[jb] = pT
                    for jb in range(njb):
                        dlt = max(0, 128 * jb - i0)
                        for half, o_ps in ((0, o_pse), (1, o_pso)):
                            nc.tensor.matmul(
                                o_ps[:, dlt:512],
                                lhsT=vaug_sb[jb][:, VW * (2 * tp + half):
                                                 VW * (2 * tp + half) + VW],
                                rhs=pTs[jb][:, 512 * half + dlt:512 * (half + 1)],
                                start=(jb == 0), stop=(jb == njb - 1),
                            )
                    # per-head-pair normalize + o^T exchange
                    osb = {}
                    for half, o_ps in ((0, o_pse), (1, o_pso)):
                        t = normp.tile([VW - 1, 512], F32, tag=f"osb{half}",
                                       name=f"osb{half}")
                        nc.scalar.copy(out=t[:], in_=o_ps[0:VW - 1, :])
                        osb[half] = t
                    sums2 = normp.tile([2, 512], F32, tag="sums2", name="sums2")
                    for half in (0, 1):
                        nc.sync.dma_start(out=sums2[half:half + 1, :],
                                          in_=osb[half][DK:DK + 1, :])
                    rec2 = normp.tile([2, 512], F32, tag="rec2", name="rec2")
                    nc.vector.reciprocal_approx_fast(out=rec2[:], in_=sums2[:])
                    stage = normp.tile([1, 512], F32, tag="stage", name="stage")
                    nc.sync.dma_start(out=stage[:], in_=rec2[1:2, :])
                    oTn = work.tile([128, 512], BF16, tag=f"oTn{tp}",
                                    name=f"oTn{tp}")
                    for half in (0, 1):
                        rep = normp.tile([DK, 512], F32, tag=f"rep{half}",
                                         name=f"rep{half}")
                        src = rec2[0:1, :] if half == 0 else stage[:]
                        nc.gpsimd.partition_broadcast(rep[:], src)
                        nc.vector.tensor_mul(
                            oTn[64 * half:64 * half + 64, :],
                            osb[half][0:DK, :], rep[:])
                    nc.gpsimd.dma_start(out=ocp[m][tp][:], in_=oTn[:])
                    nc.gpsimd.collective_compute(
                        "AllGather", mybir.AluOpType.bypass,
                        replica_groups=groups,
                        ins=[ocp[m][tp][:].opt()],
                        outs=[ocg[m][tp][:].opt()],
                    )

            def proj_chunk(m):
                i0 = 512 * m
                og = {}
                for tp in range(OT):
                    for half in (0, 1):
                        t = work.tile([128, 512], BF16, tag=f"og{tp}_{half}",
                                      name=f"og{tp}_{half}")
                        nc.gpsimd.dma_start(
                            out=t[:], in_=ocg[m][tp][128 * half:128 * (half + 1), :])
                        og[(tp, half)] = t
                korder = [(tp, half) for tp in range(OT) for half in (0, 1)]
                for rp in ((0, 1), (2, 3)):
                    yp = ps_b.tile([128, 1024], F32, tag="psb", name="yp")
                    for pos, (tp, half) in enumerate(korder):
                        kk = tp + 4 * half
                        for i, r in enumerate(rp):
                            nc.tensor.matmul(
                                yp[:, 512 * i:512 * (i + 1)],
                                lhsT=og[(tp, half)][:, 128 * r:128 * (r + 1)],
                                rhs=wo_sb[kk][:],
                                start=(pos == 0), stop=(pos == len(korder) - 1),
                            )
                    for i, r in enumerate(rp):
                        ych = work.tile([128, 512], BF16, tag="ych", name="ych")
                        nc.vector.tensor_copy(ych[:], yp[:, 512 * i:512 * (i + 1)])
                        nc.sync.dma_start(out=y[i0 + 128 * r:i0 + 128 * (r + 1), :],
                                          in_=ych[:])

            qkv_chunk(0)
            attn_chunk(0)
            qkv_chunk(1)
            proj_chunk(0)
            attn_chunk(1)
            qkv_chunk(2)
            proj_chunk(1)
            attn_chunk(2)
            qkv_chunk(3)
            proj_chunk(2)
            attn_chunk(3)
            proj_chunk(3)

    nc.compile()
    return nc


def _prep_inputs(x, Wq, Wk, Wv, Wo, cos_emb, sin_emb, token_positions):
    bf = ml_dtypes.bfloat16
    cos_g = np.asarray(cos_emb)[np.asarray(token_positions)]  # [S, DK]
    sin_g = np.asarray(sin_emb)[np.asarray(token_positions)]
    # [128, S]: partition p -> head-dim p % 64
    cosT = np.ascontiguousarray(np.tile(cos_g.T, (2, 1))).astype(bf)
    sinT = np.ascontiguousarray(np.tile(sin_g.T, (2, 1))).astype(bf)
    # rotate-half-interleaved as a matmul: rh = SWAP @ q (per 128-dim tile)
    swap = np.zeros((128, 128), np.float32)
    for j in range(64):
        swap[2 * j, 2 * j + 1] = -1.0
        swap[2 * j + 1, 2 * j] = 1.0
    swapT = np.ascontiguousarray(swap.T).astype(bf)
    # causal mask for the diagonal 128x128 block in S^T=[j,i] layout
    jj = np.arange(128)[:, None]
    ii = np.arange(128)[None, :]
    maskT = np.where(ii >= jj, 0.0, NEG).astype(np.float32)

    in_maps = []
    for c in range(NCORES):
        b, hh = c // 2, c % 2
        cols = slice(512 * hh, 512 * (hh + 1))
        in_maps.append({
            "xT": np.ascontiguousarray(np.asarray(x)[b].T).astype(bf),
            "wqT": np.ascontiguousarray(np.asarray(Wq)[cols, :].T).astype(bf),
            "wkT": np.ascontiguousarray(np.asarray(Wk)[cols, :].T).astype(bf),
            "wvT": np.ascontiguousarray(np.asarray(Wv)[cols, :].T).astype(bf),
            "woT2": np.ascontiguousarray(np.asarray(Wo)[cols, :].T).astype(bf),
            "cosT": cosT, "sinT": sinT, "swapT": swapT, "maskT": maskT,
        })
    return in_maps


def kernel(x, Wq, Wk, Wv, Wo, cos_emb, sin_emb, token_positions, **run_kwargs):
    if "nc" not in _compiled:
        _compiled["nc"] = _build_nc()
    nc = _compiled["nc"]
    in_maps = _prep_inputs(x, Wq, Wk, Wv, Wo, cos_emb, sin_emb, token_positions)
    res = run_bass_kernel_spmd(nc, in_maps, list(range(NCORES)), **run_kwargs)
    out = np.stack([
        np.concatenate([res.results[2 * b]["y"], res.results[2 * b + 1]["y"]],
                       axis=1)
        for b in range(B)
    ]).astype(np.float32)
    if run_kwargs:
        kernel.last_result = res
    return out
